# revision 13
# baseline (speedup 1.0000x reference)
"""Trainium2 Bass kernel for nn_CriticGraphPolicy (GNN message-passing critic).

Sharding: data-parallel over batch (8192 -> 8 cores x 1024), weights replicated.
Per core, activations live as [features, batch] (features on SBUF partitions):
every linear layer is a PE matmul with K = input features, N = batch columns.
Weights are host-packed into lhsT [K, M] layouts, zero-padded so K/M tiles are
128-aligned; fp32r gives 1 cycle/column matmuls at N = 512 (one PSUM bank).

Tree levels run sequentially (up: leaves->root, down: root->leaves); nodes in
a level share weights. ACT-table discipline: ops are emitted in phases so Sqrt
and Tanh (different HW activation tables) don't interleave per node.
"""

import numpy as np

import concourse.bass as bass
import concourse.mybir as mybir
import concourse.tile as tile
from concourse import bacc
from concourse.bass_utils import run_bass_kernel_spmd

F32R = mybir.dt.float32r
F32 = mybir.dt.float32
AF = mybir.ActivationFunctionType
ALU = mybir.AluOpType

NUM_LIMBS = 15
MSG = 32
STATE = 32
B = 8192
NCORES = 8
BC = B // NCORES          # batch per core = 1024
C = 512                   # column chunk (one PSUM bank of fp32)
NCH = BC // C             # chunks per node = 2
LEVELS = [[0], [1, 2], [3, 4, 5, 6], [7, 8, 9, 10, 11, 12, 13, 14]]

_BIAS_ORDER = ([("l1q", m) for m in range(7)]
               + [("l2q0", m) for m in range(3)]
               + [("l2q1", m) for m in range(3)]
               + [("l1m", m) for m in range(4)]
               + [("l2m", m) for m in range(3)]
               + [("l3m", 0), ("b1u", 0), ("b2u", 0), ("b3u", 0)])
BIDX = {k: j for j, k in enumerate(_BIAS_ORDER)}
NBIAS = len(_BIAS_ORDER)

_W_SHAPES = {
    "wq1": (65, 7, 128), "w2q1": (128, 4, 384), "w2q2": (128, 4, 384),
    "w3q": (128, 6, 2), "w1m": (64, 4, 128), "w2m": (128, 4, 384),
    "w3m": (128, 3, 64), "w1u": (33, 64), "w2u": (128, 64), "w3u": (64, 32),
    "ident": (128, 128), "ones": (64, 64), "zer": (32, 512),
}


def _st(i):
    """msg storage location of node i: (slot, partition row)."""
    return i // 4, 32 * (i % 4)


# ---------------------------------------------------------------------------
# Host-side weight packing (numpy fp32)
# ---------------------------------------------------------------------------
def _pack_weights(I):
    f32 = np.float32
    P = {}
    bias_cols = {}

    # q l1: fused [q1;q2] -> [800, 65] -> lhsT [65, 7, 128] (M padded to 896)
    w1 = np.concatenate([I["q1_l1_w"], I["q2_l1_w"]], 0)
    t = np.zeros((65, 896), f32)
    t[:, :800] = w1.T
    P["wq1"] = t.reshape(65, 7, 128)
    b1 = np.zeros(896, f32)
    b1[:800] = np.concatenate([I["q1_l1_b"], I["q2_l1_b"]])
    for m in range(7):
        bias_cols[("l1q", m)] = b1[128 * m:128 * (m + 1)]

    # l2 packer: K-tile kt reads h1 rows [128*kt - row_off, +128) of this
    # MLP's own 400 rows; M padded 300 -> 384.
    def pack_l2(wT, row_off):
        t = np.zeros((128, 4, 384), f32)
        for kt in range(4):
            base = 128 * kt - row_off
            lo = max(0, -base)
            hi = min(128, 400 - base)
            if lo < hi:
                t[lo:hi, kt, :300] = wT[base + lo:base + hi]
        return t

    P["w2q1"] = pack_l2(I["q1_l2_w"].T.astype(f32), 0)
    P["w2q2"] = pack_l2(I["q2_l2_w"].T.astype(f32), 16)
    for qi, bkey in ((0, "q1_l2_b"), (1, "q2_l2_b")):
        for m in range(3):
            v = np.zeros(128, f32)
            seg = I[bkey][128 * m:min(128 * (m + 1), 300)]
            v[:len(seg)] = seg
            if m == 2:
                v[44] = 1.0          # ones-row feeding the l3 bias
            bias_cols[(f"l2q{qi}", m)] = v

    # q l3 block-diagonal over h2q slots 0..5; bias via the ones-row (p44 of
    # slots 2 and 5)
    t = np.zeros((128, 6, 2), f32)
    for qi, (wk, bk) in enumerate((("q1_l3_w", "q1_l3_b"),
                                   ("q2_l3_w", "q2_l3_b"))):
        w3 = I[wk][0]
        for s in range(3):
            r0 = 128 * s
            n = min(128, 300 - r0)
            t[:n, 3 * qi + s, qi] = w3[r0:r0 + n]
        t[44, 3 * qi + 2, qi] = I[bk][0]
    P["w3q"] = t

    # mb l1: [400, 64] -> [64, 4, 128] (M padded 512)
    t = np.zeros((64, 512), f32)
    t[:, :400] = I["mb_l1_w"].T
    P["w1m"] = t.reshape(64, 4, 128)
    bmb1 = np.zeros(512, f32)
    bmb1[:400] = I["mb_l1_b"]
    for m in range(4):
        bias_cols[("l1m", m)] = bmb1[128 * m:128 * (m + 1)]

    # mb l2 (no ones-row needed)
    P["w2m"] = pack_l2(I["mb_l2_w"].T.astype(f32), 0)
    for m in range(3):
        v = np.zeros(128, f32)
        seg = I["mb_l2_b"][128 * m:min(128 * (m + 1), 300)]
        v[:len(seg)] = seg
        bias_cols[("l2m", m)] = v

    # mb l3: K = 300 (3 tiles), M = 64
    t = np.zeros((128, 3, 64), f32)
    w3m = I["mb_l3_w"].T
    for kt in range(3):
        r0 = 128 * kt
        n = min(128, 300 - r0)
        t[:n, kt, :] = w3m[r0:r0 + n]
    P["w3m"] = t
    v = np.zeros(128, f32)
    v[:64] = I["mb_l3_b"]
    bias_cols[("l3m", 0)] = v

    # up weights
    P["w1u"] = I["up_fc1_w"].T.astype(f32).copy()
    P["w2u"] = I["up_fc2_w"].T.astype(f32).copy()
    P["w3u"] = I["up_fc3_w"].T.astype(f32).copy()
    for key, bk, n in (("b1u", "up_fc1_b", 64), ("b2u", "up_fc2_b", 64),
                       ("b3u", "up_fc3_b", 32)):
        v = np.zeros(128, f32)
        v[:n] = I[bk]
        bias_cols[(key, 0)] = v

    bm = np.zeros((128, NBIAS), f32)
    for k, j in BIDX.items():
        bm[:, j] = bias_cols[k]
    P["biases"] = bm
    P["ident"] = np.eye(128, dtype=f32)
    P["ones"] = np.ones((64, 64), f32)
    P["zer"] = np.zeros((32, 512), f32)
    return P


# ---------------------------------------------------------------------------
# Device kernel
# ---------------------------------------------------------------------------
def build_nc(repeat=1):
    nc = bacc.Bacc("TRN2", target_bir_lowering=False, debug=False)
    x_d = nc.dram_tensor("x", [NUM_LIMBS, BC // 128, 128, STATE], F32R,
                         kind="ExternalInput")
    u_d = nc.dram_tensor("u", [NUM_LIMBS, BC], F32R, kind="ExternalInput")
    wd = {k: nc.dram_tensor(k, list(s), F32R, kind="ExternalInput")
          for k, s in _W_SHAPES.items()}
    bias_d = nc.dram_tensor("biases", [128, NBIAS], F32R, kind="ExternalInput")
    q_d = nc.dram_tensor("q", [NUM_LIMBS, 2, BC], F32, kind="ExternalOutput")

    with tile.TileContext(nc) as tc:
        with (
            tc.tile_pool(name="const", bufs=1) as const,
            tc.tile_pool(name="msg", bufs=1) as msgp,
            tc.tile_pool(name="cross", bufs=5) as cross,
            tc.tile_pool(name="trans", bufs=2) as trans,
            tc.tile_pool(name="big", bufs=1) as big,
            tc.tile_pool(name="ps", bufs=3, space=bass.MemorySpace.PSUM) as ps,
            tc.tile_pool(name="ps2", bufs=3, space=bass.MemorySpace.PSUM) as ps2,
            tc.tile_pool(name="ps3", bufs=2, space=bass.MemorySpace.PSUM) as ps3,
        ):
            W = {k: const.tile(list(s), F32R, name=k, tag=k)
                 for k, s in _W_SHAPES.items()}
            for k in _W_SHAPES:
                nc.sync.dma_start(out=W[k][...], in_=wd[k].ap())
            bias = const.tile([128, NBIAS], F32R, tag="bias")
            nc.sync.dma_start(out=bias[:, :], in_=bias_d[:, :])
            ident = W["ident"]
            ones = W["ones"]
            zer = W["zer"]

            msgup = msgp.tile([128, 4, BC], F32R, tag="msgup")
            msgin = msgp.tile([128, 4, BC], F32R, tag="msgin")

            pools = (cross, trans, big, ps, ps2, ps3, zer)
            for _ in range(repeat):
                _one_pass(nc, pools, W, bias, ident, ones, msgup, msgin,
                          x_d, u_d, q_d)
    nc.compile()
    return nc


def _one_pass(nc, pools, W, bias, ident, ones, msgup, msgin, x_d, u_d, q_d):
    cross, trans, big, ps, ps2, ps3, zer = pools

    def bias_ap(key, p=128):
        return bias[0:p, BIDX[key]:BIDX[key] + 1]

    def evict_relu(dst, src, bkey, eng):
        if eng == "s":
            nc.scalar.activation(dst, src, AF.Relu, bias=bias_ap(bkey))
        else:
            nc.vector.tensor_scalar(dst, src, bias_ap(bkey).bitcast(F32),
                                    0.0, ALU.add, ALU.max)

    # ================= UP PASS =================
    for lvl in (3, 2, 1, 0):
        nodes = LEVELS[lvl]
        for g0 in range(0, len(nodes), 2):
            grp = nodes[g0:g0 + 2]
            st = {}
            # --- phase A (sqrt-table ACT ops): x load/transpose, fc1, norm
            for i in grp:
                for ch in range(NCH):
                    cols = slice(ch * C, (ch + 1) * C)
                    x_t = trans.tile([128, 4, STATE], F32R, tag="x_t")
                    nc.sync.dma_start(
                        out=x_t[...],
                        in_=x_d[i, 4 * ch:4 * ch + 4, :, :].rearrange(
                            "j p f -> p j f"))
                    xt_ps = ps3.tile([32, 512], F32R, tag="pst")
                    for j in range(4):
                        nc.tensor.transpose(xt_ps[:, 128 * j:128 * (j + 1)],
                                            x_t[:, j, :], ident[:, :])
                    xin = trans.tile([33, C], F32R, tag="xin")
                    nc.vector.tensor_copy(xin[0:32, :], xt_ps[:, :])
                    nc.sync.dma_start(out=xin[32:33, :], in_=u_d[i:i + 1, cols])
                    p1 = ps2.tile([64, C], F32, tag="psm")
                    nc.tensor.matmul(p1[:, :], W["w1u"][:, :], xin[:, :],
                                     start=True, stop=True)
                    xu_b = cross.tile([64, C], F32R, tag="xu_b")
                    nc.scalar.activation(xu_b[:, :], p1[:, :], AF.Identity,
                                         bias=bias_ap(("b1u", 0), 64))
                    sq = trans.tile([64, C], F32R, tag="sqs")
                    nc.gpsimd.tensor_mul(sq[:, :], xu_b[:, :], xu_b[:, :])
                    p2 = ps2.tile([64, C], F32, tag="psm")
                    nc.tensor.matmul(p2[:, :], ones[:, :], sq[:, :],
                                     start=True, stop=True)
                    nrm = cross.tile([64, C], F32R, tag="nrm")
                    nc.scalar.activation(nrm[:, :], p2[:, :], AF.Sqrt)
                    st[(i, ch)] = (xu_b, nrm)
            # --- phase B (tanh-table): normalize-mul, h assembly, fc2, fc3
            for i in grp:
                for ch in range(NCH):
                    cols = slice(ch * C, (ch + 1) * C)
                    xu_b, nrm = st[(i, ch)]
                    rcp = trans.tile([64, C], F32R, tag="rcps")
                    nc.vector.reciprocal_approx_fast(rcp[:, :].bitcast(F32), nrm[:, :].bitcast(F32))
                    h_up = trans.tile([128, C], F32R, tag="h_up")
                    nc.vector.tensor_mul(h_up[0:64, :], xu_b[:, :], rcp[:, :])
                    if lvl < 3:
                        for k, chd in enumerate((2 * i + 1, 2 * i + 2)):
                            s, p = _st(chd)
                            nc.sync.dma_start(
                                out=h_up[64 + 32 * k:96 + 32 * k, :],
                                in_=msgup[p:p + 32, s, cols])
                        nc.scalar.activation(h_up[:, :], h_up[:, :], AF.Tanh)
                        kk = 128
                    else:
                        nc.scalar.activation(h_up[0:64, :], h_up[0:64, :],
                                             AF.Tanh)
                        kk = 64
                    p3 = ps2.tile([64, C], F32, tag="psm")
                    nc.tensor.matmul(p3[:, :], W["w2u"][0:kk, :],
                                     h_up[0:kk, :], start=True, stop=True)
                    h2u = trans.tile([64, C], F32R, tag="h2u")
                    nc.scalar.activation(h2u[:, :], p3[:, :], AF.Tanh,
                                         bias=bias_ap(("b2u", 0), 64))
                    p4 = ps2.tile([32, C], F32, tag="psm")
                    nc.tensor.matmul(p4[:, :], W["w3u"][:, :], h2u[:, :],
                                     start=True, stop=True)
                    m_b = cross.tile([32, C], F32R, tag="m_b")
                    nc.scalar.activation(m_b[:, :], p4[:, :], AF.Identity,
                                         bias=bias_ap(("b3u", 0), 32))
                    st[(i, ch)] = m_b
            # --- phase C (sqrt-table): msgup normalize + store
            for i in grp:
                for ch in range(NCH):
                    cols = slice(ch * C, (ch + 1) * C)
                    m_b = st.pop((i, ch))
                    sq2 = trans.tile([32, C], F32R, tag="sqs")
                    nc.gpsimd.tensor_mul(sq2[:, :], m_b[:, :], m_b[:, :])
                    p5 = ps2.tile([32, C], F32, tag="psm")
                    nc.tensor.matmul(p5[:, :], ones[0:32, 0:32], sq2[:, :],
                                     start=True, stop=True)
                    nrm2 = trans.tile([32, C], F32R, tag="nrms")
                    nc.scalar.activation(nrm2[:, :], p5[:, :], AF.Sqrt)
                    rcp2 = trans.tile([32, C], F32R, tag="rcps")
                    nc.vector.reciprocal_approx_fast(rcp2[:, :].bitcast(F32), nrm2[:, :].bitcast(F32))
                    mun = trans.tile([32, C], F32R, tag="mn")
                    nc.vector.tensor_mul(mun[:, :], m_b[:, :], rcp2[:, :])
                    s, p = _st(i)
                    nc.sync.dma_start(out=msgup[p:p + 32, s, cols],
                                      in_=mun[:, :])

    # ================= DOWN PASS =================
    for lvl in range(4):
        nodes = LEVELS[lvl]
        has_mb = lvl < 3
        for g0 in range(0, len(nodes), 2):
            grp = nodes[g0:g0 + 2]
            st = {}
            # --- phase X (tanh-table): assemble xum/xm
            for i in grp:
                s, p = _st(i)
                for ch in range(NCH):
                    cols = slice(ch * C, (ch + 1) * C)
                    xum = cross.tile([65, C], F32R, tag="xum")
                    nc.sync.dma_start(out=xum[0:32, :],
                                      in_=msgup[p:p + 32, s, cols])
                    if lvl > 0:
                        nc.sync.dma_start(out=xum[33:65, :],
                                          in_=msgin[p:p + 32, s, cols])
                    else:
                        nc.sync.dma_start(out=xum[33:65, :], in_=zer[:, :])
                    nc.sync.dma_start(out=xum[32:33, :],
                                      in_=u_d[i:i + 1, cols])
                    xm = None
                    if has_mb:
                        xm = cross.tile([64, C], F32R, tag="xm")
                        nc.sync.dma_start(out=xm[0:32, :],
                                          in_=msgup[p:p + 32, s, cols])
                        if lvl > 0:
                            nc.sync.dma_start(out=xm[32:64, :],
                                              in_=msgin[p:p + 32, s, cols])
                        else:
                            nc.sync.dma_start(out=xm[32:64, :], in_=zer[:, :])
                        nc.scalar.activation(xm[:, :], xm[:, :], AF.Tanh)
                    st[(i, ch)] = (xum, xm)
            # --- phase Y (sqrt-table): the MLPs
            for i in grp:
                for ch in range(NCH):
                    cols = slice(ch * C, (ch + 1) * C)
                    xum, xm = st.pop((i, ch))
                    h1q = big.tile([128, 7, C], F32R, tag="h1q")
                    for m in range(7):
                        pl1 = ps.tile([128, C], F32, tag="pmm")
                        nc.tensor.matmul(pl1[:, :], W["wq1"][:, m, :],
                                         xum[:, :], start=True, stop=True)
                        evict_relu(h1q[:, m, :], pl1[:, :], ("l1q", m),
                                   "s" if m < 3 else "v")
                    h2q = big.tile([128, 6, C], F32R, tag="h2q")
                    for qi, wk in ((0, "w2q1"), (1, "w2q2")):
                        for m in range(3):
                            pl2 = ps.tile([128, C], F32, tag="pmm")
                            for kt in range(4):
                                nc.tensor.matmul(
                                    pl2[:, :],
                                    W[wk][:, kt, 128 * m:128 * (m + 1)],
                                    h1q[:, (kt if qi == 0 else 3 + kt), :],
                                    start=(kt == 0), stop=(kt == 3))
                            evict_relu(h2q[:, 3 * qi + m, :], pl2[:, :],
                                       (f"l2q{qi}", m), "s" if m == 0 else "v")
                    pl3 = ps2.tile([2, C], F32, tag="psm")
                    for kt in range(6):
                        nc.tensor.matmul(pl3[:, :], W["w3q"][:, kt, :],
                                         h2q[:, kt, :],
                                         start=(kt == 0), stop=(kt == 5))
                    q_sb = trans.tile([2, C], F32, tag="q_sb")
                    nc.vector.tensor_copy(q_sb[:, :], pl3[:, :])
                    nc.sync.dma_start(out=q_d[i, :, cols], in_=q_sb[:, :])

                    if has_mb:
                        h1m = big.tile([128, 4, C], F32R, tag="h1m")
                        for m in range(4):
                            pm1 = ps.tile([128, C], F32, tag="pmm")
                            nc.tensor.matmul(pm1[:, :], W["w1m"][:, m, :],
                                             xm[:, :], start=True, stop=True)
                            evict_relu(h1m[:, m, :], pm1[:, :], ("l1m", m),
                                       "s" if m < 2 else "v")
                        h2m = big.tile([128, 3, C], F32R, tag="h2m")
                        for m in range(3):
                            pm2 = ps.tile([128, C], F32, tag="pmm")
                            for kt in range(4):
                                nc.tensor.matmul(
                                    pm2[:, :],
                                    W["w2m"][:, kt, 128 * m:128 * (m + 1)],
                                    h1m[:, kt, :],
                                    start=(kt == 0), stop=(kt == 3))
                            evict_relu(h2m[:, m, :], pm2[:, :], ("l2m", m),
                                       "s" if m == 0 else "v")
                        pm3 = ps2.tile([64, C], F32, tag="psm")
                        for kt in range(3):
                            nc.tensor.matmul(pm3[:, :], W["w3m"][:, kt, :],
                                             h2m[:, kt, :],
                                             start=(kt == 0), stop=(kt == 2))
                        md = trans.tile([64, C], F32R, tag="md")
                        nc.scalar.activation(md[:, :], pm3[:, :], AF.Identity,
                                             bias=bias_ap(("l3m", 0), 64))
                        sq3 = trans.tile([64, C], F32R, tag="sqs")
                        nc.gpsimd.tensor_mul(sq3[:, :], md[:, :], md[:, :])
                        pn = ps2.tile([64, C], F32, tag="psm")
                        nc.tensor.matmul(pn[:, :], ones[:, :], sq3[:, :],
                                         start=True, stop=True)
                        nrm3 = trans.tile([64, C], F32R, tag="nrms")
                        nc.scalar.activation(nrm3[:, :], pn[:, :], AF.Sqrt)
                        rcp3 = trans.tile([64, C], F32R, tag="rcps")
                        nc.vector.reciprocal_approx_fast(rcp3[:, :].bitcast(F32), nrm3[:, :].bitcast(F32))
                        mdn = trans.tile([64, C], F32R, tag="mn")
                        nc.vector.tensor_mul(mdn[:, :], md[:, :], rcp3[:, :])
                        for k, chd in enumerate((2 * i + 1, 2 * i + 2)):
                            s2, p2_ = _st(chd)
                            nc.sync.dma_start(
                                out=msgin[p2_:p2_ + 32, s2, cols],
                                in_=mdn[32 * k:32 * (k + 1), :])


# ---------------------------------------------------------------------------
# Public entry point
# ---------------------------------------------------------------------------
_CACHE = {}


def get_nc(repeat=1):
    if repeat not in _CACHE:
        _CACHE[repeat] = build_nc(repeat)
    return _CACHE[repeat]


def make_in_maps(**inputs):
    P = _pack_weights(inputs)
    x = np.asarray(inputs["x"], np.float32)
    u = np.asarray(inputs["u"], np.float32)
    in_maps = []
    for c in range(NCORES):
        sl = slice(c * BC, (c + 1) * BC)
        m = {k: np.ascontiguousarray(P[k]) for k in _W_SHAPES}
        m["biases"] = np.ascontiguousarray(P["biases"])
        m["x"] = np.ascontiguousarray(
            x[:, sl, :].reshape(NUM_LIMBS, BC // 128, 128, STATE))
        m["u"] = np.ascontiguousarray(u[:, sl, 0])
        in_maps.append(m)
    return in_maps


def kernel(**inputs):
    in_maps = make_in_maps(**inputs)
    nc = get_nc()
    res = run_bass_kernel_spmd(nc, in_maps, core_ids=list(range(NCORES)))
    out = np.empty((NUM_LIMBS, B, 2), np.float32)
    for c in range(NCORES):
        out[:, c * BC:(c + 1) * BC, :] = res.results[c]["q"].transpose(0, 2, 1)
    return out
